# revision 1
# baseline (speedup 1.0000x reference)
"""Multi-head attention (QKV proj + rotary + softmax attention + out proj)
for Trainium2, sharded over 8 NeuronCores.

Problem: x[2,2048,1024], 16 heads x dh=64, rotary embedding, softmax
attention, output projection + bias.

Sharding: batch x head-group. Core c handles batch c//4 and the 4 heads
[4*(c%4), 4*(c%4)+4). Each core computes its QKV slice, rotary, attention,
and a partial output projection; the host sums the 4 partial projections
per batch and adds the bias.

Device-side design (per core, everything in "transposed" layout):
  - qkvT = W @ x^T computed as f32r matmuls (full PE rate, tf32-ish
    precision): qT/kT produced as [dh-pair(128), n] tiles, v as natural
    [n, e] tiles.
  - rotary applied on the fp32 psum output via DVE: q*cos +
    pairswap(q*sin_pre), with the dh dimension stored interleaved
    ([0,32,1,33,...]) so rotate_half becomes an adjacent-lane
    stream_shuffle. Output f32r.
  - dots: scoresT[j,n] = krotT^T-slice @ qrotT, two heads packed in the
    128x128 PE array via tile_position row-tiling (K=64 each). fp32 psum.
  - softmax without max-subtraction (logits are O(+-6)): ACT exp over
    2-j-tile psum batches (N=1024 per ACTIVATE), output fp16.
  - AV: lhsT = [v | ones] (M=65, fp16) so row 64 accumulates the softmax
    denominators for free; fp32 psum accumulation over the 16 j-tiles.
  - normalize: reciprocal_approx_fast of the sums row, partition-broadcast
    via a DRAM round-trip DMA (K=1 ones-matmul on the tail block), one DVE
    multiply -> aoT (f32r).
  - output proj: y[n,d] accumulated over the two head-pair e-chunks, f32r;
    the last block's pair-0 partial goes to a separate output (y3a) summed
    on the host, so the kernel tail only runs the pair-1 projection.
"""
import sys

sys.path.insert(0, "/opt/trn_rl_repo")

import numpy as np

import concourse.bacc as bacc
import concourse.tile as tile
from concourse import mybir
from concourse.bass_utils import run_bass_kernel_spmd

F32 = mybir.dt.float32
F32R = mybir.dt.float32r
BF16 = mybir.dt.bfloat16
FP16 = mybir.dt.float16
EXP = mybir.ActivationFunctionType.Exp
MULT = mybir.AluOpType.mult
ADD = mybir.AluOpType.add

B, N, DIM = 2, 2048, 1024
H, DH = 16, 64
INNER = H * DH
SCALE = DH ** -0.5
NCORES = 8
HPC = H // (NCORES // B)      # heads per core = 4
NPAIR = HPC // 2              # head pairs per core = 2

P = 128
NT = N // 512                 # 4 n-tiles of 512
DC = DIM // P                 # 8 d-chunks
JTILES = N // P               # 16 j-tiles
JB = JTILES // 2              # 8 j-batches (2 j-tiles each)

PAIRSWAP = [i ^ 1 for i in range(32)]

_CACHE = {}


def _build():
    nc = bacc.Bacc(None, target_bir_lowering=False, debug=False)
    with tile.TileContext(nc) as tc:
        with tc.tile_pool(name="dram", bufs=1, space="DRAM") as dram, \
             tc.tile_pool(name="const", bufs=1) as const, \
             tc.tile_pool(name="perst", bufs=1) as perst, \
             tc.tile_pool(name="tmp", bufs=1) as tmp, \
             tc.tile_pool(name="ps", bufs=1, space="PSUM") as ps:
            # ---------------- DRAM I/O ----------------
            xT_d = dram.tile([DIM, N], F32R, kind="ExternalInput", name="xT", uniquify=False)
            wqkT_d = dram.tile([DIM, 512], F32R, kind="ExternalInput", name="wqkT", uniquify=False)
            wvT_d = dram.tile([DIM, 256], F32R, kind="ExternalInput", name="wvT", uniquify=False)
            cq_d = dram.tile([P, N], F32, kind="ExternalInput", name="cq", uniquify=False)
            sq_d = dram.tile([P, N], F32, kind="ExternalInput", name="sq", uniquify=False)
            ck_d = dram.tile([P, N], F32, kind="ExternalInput", name="ck", uniquify=False)
            sk_d = dram.tile([P, N], F32, kind="ExternalInput", name="sk", uniquify=False)
            woT_d = dram.tile([256, DIM], F32R, kind="ExternalInput", name="woT", uniquify=False)
            y_d = dram.tile([N, DIM], F32, kind="ExternalOutput", name="y", uniquify=False)
            y3a_d = dram.tile([512, DIM], F32, kind="ExternalOutput", name="y3a", uniquify=False)

            # ---------------- constants to SBUF ----------------
            wqk_r = wqkT_d.rearrange("(c p) e -> p c e", p=P)
            wqk_sb = []
            for ech in (2, 0, 3, 1):    # k0, q0, k1, q1 arrival order
                w = const.tile([P, DC, P], F32R, name=f"wqk{ech}")
                nc.sync.dma_start(w[:, :, :], wqk_r[:, :, ech * P:(ech + 1) * P])
                wqk_sb.append((ech, w))
            wqk_sb = [w for _, w in sorted(wqk_sb)]
            wv_sb = const.tile([P, DC, 256], F32R)
            nc.sync.dma_start(wv_sb[:, :, :], wvT_d.rearrange("(c p) e -> p c e", p=P))
            wo_sb = const.tile([P, NPAIR, DIM], F32R)
            nc.sync.dma_start(wo_sb[:, :, :], woT_d.rearrange("(c p) d -> p c d", p=P))
            cq_sb = const.tile([P, N], F32)
            nc.sync.dma_start(cq_sb[:, :], cq_d[:, :])
            sq_sb = const.tile([P, N], F32)
            nc.sync.dma_start(sq_sb[:, :], sq_d[:, :])
            ck_sb = const.tile([P, N], F32)
            nc.sync.dma_start(ck_sb[:, :], ck_d[:, :])
            sk_sb = const.tile([P, N], F32)
            nc.sync.dma_start(sk_sb[:, :], sk_d[:, :])

            ones_f = const.tile([1, 64], F32)
            nc.vector.memset(ones_f[:, :], 1.0)
            ones_r = const.tile([1, 64], F32R)
            nc.vector.tensor_copy(ones_r[:, :], ones_f[:, :])

            # ---------------- persistent tiles ----------------
            qrot = [[perst.tile([P, 512], F32R, name=f"qrot{p}_{t}")
                     for t in range(NT)] for p in range(NPAIR)]
            krot = [[perst.tile([P, 512], F32R, name=f"krot{p}_{t}")
                     for t in range(NT)] for p in range(NPAIR)]
            v_aug = [perst.tile([P, 4, HPC, 65], FP16, name=f"vaug{t}")
                     for t in range(NT)]
            for t in range(NT):
                nc.vector.memset(v_aug[t][:, :, :, 64:65], 1.0)
            aoT = [[perst.tile([P, 512], F32R, name=f"aoT{p}_{t}")
                    for t in range(NT)] for p in range(NPAIR)]

            # ---------------- helpers ----------------
            xT_r = xT_d.rearrange("(c p) n -> p c n", p=P)

            def load_x(t):
                # per-d-chunk tiles so matmuls can start as soon as the first
                # 256KB chunk lands instead of waiting for the full 2MB tile
                xt = [tmp.tile([P, 512], F32R, name=f"xt{c}", tag=f"xt{c}", bufs=2)
                      for c in range(DC)]
                for c in range(DC):
                    nc.sync.dma_start(xt[c][:, :], xT_r[:, c, t * 512:(t + 1) * 512])
                return xt

            def qk_chunk(ech, t, xt, dest, cos_sb, sin_sb):
                # qkvT e-chunk [128, 512] = W-chunk @ xT-tile, then rotary.
                pqk = ps.tile([P, 512], F32, name="pqk", tag="m", bufs=2)
                for c in range(DC):
                    nc.tensor.matmul(pqk[:, :],
                                     wqk_sb[ech][:, c, :],
                                     xt[c][:, :],
                                     start=(c == 0), stop=(c == DC - 1))
                sl = slice(t * 512, (t + 1) * 512)
                t1 = tmp.tile([P, 512], F32, name="t1", tag="t1", bufs=2)
                t2 = tmp.tile([P, 512], F32, name="t2", tag="t2", bufs=2)
                t3 = tmp.tile([P, 512], F32, name="t3", tag="t3", bufs=2)
                nc.vector.tensor_tensor(t1[:, :], pqk[:, :], cos_sb[:, sl], op=MULT)
                nc.vector.tensor_tensor(t2[:, :], pqk[:, :], sin_sb[:, sl], op=MULT)
                nc.vector.stream_shuffle(t3[:, :], t2[:, :], PAIRSWAP)
                nc.vector.tensor_tensor(dest[:, :], t1[:, :], t3[:, :], op=ADD)

            def v_tile(t, xt):
                # v natural [n, e] for the 4 local heads, by 128-row subtiles
                for nsub in range(4):
                    pv = ps.tile([P, 256], F32, name="pv", tag="m", bufs=2)
                    for c in range(DC):
                        nc.tensor.matmul(pv[:, :],
                                         xt[c][:, nsub * P:(nsub + 1) * P],
                                         wv_sb[:, c, :],
                                         start=(c == 0), stop=(c == DC - 1))
                    nc.vector.tensor_copy(
                        v_aug[t][:, nsub, :, 0:64],
                        pv[:, :].rearrange("p (h d) -> p h d", h=HPC))

            def qkv_for_tile(t, ops):
                xt = load_x(t)
                for op in ops:
                    if op == "k0":
                        qk_chunk(2, t, xt, krot[0][t], ck_sb, sk_sb)
                    elif op == "k1":
                        qk_chunk(3, t, xt, krot[1][t], ck_sb, sk_sb)
                    elif op == "q0":
                        qk_chunk(0, t, xt, qrot[0][t], cq_sb, sq_sb)
                    elif op == "q1":
                        qk_chunk(1, t, xt, qrot[1][t], cq_sb, sq_sb)
                    elif op == "v":
                        v_tile(t, xt)

            def attention(nq, pair, pre_jb=None, mid_jb=None):
                pav = [ps.tile([65, 512], F32, name=f"pav{h}", tag="av", bufs=2)
                       for h in range(2)]
                for jb in range(JB):
                    if pre_jb is not None:
                        pre_jb(jb)
                    sc = [ps.tile([P, 2, 512], F32, name=f"sc{h}", tag="s", bufs=2)
                          for h in range(2)]
                    for jl in range(2):
                        jt = jb * 2 + jl
                        kt = krot[pair][jt // 4]
                        jsl = slice((jt % 4) * P, (jt % 4 + 1) * P)
                        for h in range(2):
                            rows = slice(h * 64, (h + 1) * 64)
                            nc.tensor.matmul(sc[h][:, jl, :],
                                             kt[rows, jsl],
                                             qrot[pair][nq][rows, :],
                                             start=True, stop=True,
                                             tile_position=(h * 64, 0))
                    ex = [tmp.tile([P, 2, 512], FP16, name=f"ex{h}", tag="ex", bufs=4)
                          for h in range(2)]
                    for h in range(2):
                        nc.scalar.activation(ex[h][:, :, :], sc[h][:, :, :], EXP)
                    if mid_jb is not None:
                        mid_jb(jb)
                    for jl in range(2):
                        jt = jb * 2 + jl
                        for h in range(2):
                            nc.tensor.matmul(pav[h][:, :],
                                             v_aug[jt // 4][:, jt % 4, pair * 2 + h, :],
                                             ex[h][:, jl, :],
                                             start=(jt == 0), stop=(jt == JTILES - 1))
                for h in range(2):
                    # evacuate psum immediately so the next (nq, pair) can start;
                    # sums row copied separately so it lands at partition 0
                    # (custom-DVE reciprocal_approx_fast requires base_partition 0)
                    av_sb = tmp.tile([64, 512], F32, name="av_sb", tag="avs", bufs=3)
                    sm_sb = tmp.tile([1, 512], F32, name="sm_sb", tag="sms", bufs=4)
                    nc.vector.tensor_copy(av_sb[:, :], pav[h][0:64, :])
                    nc.vector.tensor_copy(sm_sb[:, :], pav[h][64:65, :])
                    rc = tmp.tile([1, 512], F32, name="rc", tag="rc", bufs=2)
                    nc.vector.reciprocal_approx_fast(rc[:, :], sm_sb[:, :])
                    bc = tmp.tile([64, 512], F32, name="bc", tag="bc", bufs=2)
                    if nq == NT - 1:
                        # tail-critical: broadcast via K=1 ones-matmul (no DMA
                        # round-trip latency before the last y projection)
                        rcr = tmp.tile([1, 512], F32R, name="rcr", tag="rcr", bufs=2)
                        nc.vector.tensor_copy(rcr[:, :], rc[:, :])
                        pbc = ps.tile([64, 512], F32, name="pbc", tag="m", bufs=2)
                        nc.tensor.matmul(pbc[:, :], ones_r[:, :], rcr[:, :],
                                         start=True, stop=True)
                        nc.vector.tensor_copy(bc[:, :], pbc[:, :])
                    else:
                        # broadcast across partitions via a DRAM round-trip
                        rd = dram.tile([1, 512], F32, name="rd", tag="rd", bufs=2)
                        nc.sync.dma_start(rd[:, :], rc[:, :])
                        nc.sync.dma_start(bc[:, :], rd.to_broadcast([64, 512]))
                    rows = slice(h * 64, (h + 1) * 64)
                    nc.vector.tensor_tensor(aoT[pair][nq][rows, :],
                                            av_sb[:, :], bc[:, :], op=MULT)

            def y_proj_pair(nq, pair, out_d, row0):
                # single-pair partial projection (no cross-pair accumulation)
                for nsub in range(4):
                    ys = tmp.tile([P, DIM], F32, name="ysp", tag="ys", bufs=2)
                    nsl = slice(nsub * P, (nsub + 1) * P)
                    for dh2 in range(2):
                        py = ps.tile([P, 512], F32, name="pyp", tag="m", bufs=2)
                        dsl = slice(dh2 * 512, (dh2 + 1) * 512)
                        nc.tensor.matmul(py[:, :], aoT[pair][nq][:, nsl],
                                         wo_sb[:, pair, dsl],
                                         start=True, stop=True)
                        nc.vector.tensor_copy(ys[:, dsl], py[:, :])
                    nc.sync.dma_start(out_d[row0 + nsub * P:row0 + (nsub + 1) * P, :],
                                      ys[:, :])

            def y_proj(nq):
                for nsub in range(4):
                    ys = tmp.tile([P, DIM], F32, name="ys", tag="ys", bufs=2)
                    nsl = slice(nsub * P, (nsub + 1) * P)
                    for dh2 in range(2):
                        py = ps.tile([P, 512], F32, name="py", tag="m", bufs=2)
                        dsl = slice(dh2 * 512, (dh2 + 1) * 512)
                        for pair in range(NPAIR):
                            nc.tensor.matmul(py[:, :],
                                             aoT[pair][nq][:, nsl],
                                             wo_sb[:, pair, dsl],
                                             start=(pair == 0), stop=(pair == NPAIR - 1))
                        nc.vector.tensor_copy(ys[:, dsl], py[:, :])
                    nc.sync.dma_start(y_d[nq * 512 + nsub * P:
                                          nq * 512 + (nsub + 1) * P, :], ys[:, :])

            # ---------------- emission order ----------------
            # Tile has sequential program-order semantics: every tile must be
            # written (in emission order) before anything that reads it, and
            # per-psum-tag slot reuse is FIFO in emission order. QKV work and
            # the previous block's output projection are threaded just-in-time
            # through the attention j-loops: k before the dots that need it,
            # v between exp and the AV that needs it, next-q early (ahead of
            # y in the shared psum-tag FIFO) so rotary completes before the
            # block boundary.
            qkv_for_tile(0, ["k0", "q0"])

            def pre_first(jb):
                if jb == 1:
                    qkv_for_tile(0, ["k1", "q1"])
                elif jb in (2, 4, 6):
                    qkv_for_tile(jb // 2, ["k0", "k1"])

            def mid_first(jb):
                if jb in (0, 2, 4, 6):
                    qkv_for_tile(jb // 2, ["v"])

            def make_pre_q(t):
                def pre(jb):
                    if jb == 1:
                        qkv_for_tile(t, ["q0", "q1"])
                return pre

            def make_pre(nq):
                def pre(jb):
                    if jb == 1 and nq + 1 < NT:
                        qkv_for_tile(nq + 1, ["q0", "q1"])
                    if jb == 4 and nq >= 1:
                        y_proj(nq - 1)
                return pre

            for nq in range(NT):
                for pair in range(NPAIR):
                    if nq == 0 and pair == 0:
                        attention(nq, pair, pre_jb=pre_first, mid_jb=mid_first)
                    elif nq == 0 and pair == 1:
                        attention(nq, pair, pre_jb=make_pre_q(1))
                    elif pair == 0:
                        attention(nq, pair, pre_jb=make_pre(nq))
                    elif nq == NT - 1:
                        def pre_y3a(jb):
                            if jb == 2:
                                y_proj_pair(NT - 1, 0, y3a_d, 0)
                        attention(nq, pair, pre_jb=pre_y3a)
                    else:
                        attention(nq, pair)
            y_proj_pair(NT - 1, 1, y_d, (NT - 1) * 512)
    nc.compile()
    return nc


def _host_prep(x, rotary_emb, w_qkv, w_out):
    """Build the 8 per-core input maps."""
    x = np.asarray(x, dtype=np.float32)
    rotary_emb = np.asarray(rotary_emb, dtype=np.float32)
    w_qkv = np.asarray(w_qkv, dtype=np.float32)
    w_out = np.asarray(w_out, dtype=np.float32)

    # interleaved dh permutation: new row 2i <- dim i, 2i+1 <- dim 32+i
    perm = np.empty(DH, dtype=np.int64)
    perm[0::2] = np.arange(32)
    perm[1::2] = np.arange(32) + 32
    pair_swap = np.arange(DH) ^ 1

    cos = np.cos(rotary_emb).T[perm]                      # [dh, n] permuted
    sin = np.sin(rotary_emb).T[perm]
    sign = np.where(perm < 32, -1.0, 1.0)[:, None].astype(np.float32)
    sin_eff = sign * sin
    sin_pre = sin_eff[pair_swap]                          # pre-swapped
    c2 = np.concatenate([cos, cos], axis=0)               # [128, n]
    s2 = np.concatenate([sin_pre, sin_pre], axis=0)
    cq = np.ascontiguousarray(SCALE * c2)
    sq = np.ascontiguousarray(SCALE * s2)
    ck = np.ascontiguousarray(c2)
    sk = np.ascontiguousarray(s2)

    in_maps = []
    for core in range(NCORES):
        b = core // (NCORES // B)
        g = core % (NCORES // B)
        heads = range(4 * g, 4 * g + HPC)
        q_rows = np.concatenate([h * DH + perm for h in heads])
        k_rows = np.concatenate([INNER + h * DH + perm for h in heads])
        v_rows = np.arange(2 * INNER + 4 * g * DH, 2 * INNER + (4 * g + HPC) * DH)
        wqkT = np.ascontiguousarray(w_qkv[np.concatenate([q_rows, k_rows])].T)
        wvT = np.ascontiguousarray(w_qkv[v_rows].T)
        woT = np.ascontiguousarray(w_out[:, 4 * g * DH:(4 * g + HPC) * DH].T)
        xT = np.ascontiguousarray(x[b].T)
        in_maps.append({
            "xT": xT, "wqkT": wqkT, "wvT": wvT,
            "cq": cq, "sq": sq, "ck": ck, "sk": sk, "woT": woT,
        })
    return in_maps


def kernel(x, rotary_emb, w_qkv, w_out, b_out, _trace=False):
    if "nc" not in _CACHE:
        _CACHE["nc"] = _build()
    nc = _CACHE["nc"]
    in_maps = _host_prep(x, rotary_emb, w_qkv, w_out)
    res = run_bass_kernel_spmd(nc, in_maps, core_ids=list(range(NCORES)),
                               trace=_trace)
    _CACHE["last_result"] = res
    y = np.zeros((B, N, DIM), dtype=np.float32)
    for core in range(NCORES):
        b = core // (NCORES // B)
        y[b] += res.results[core]["y"]
        y[b, (NT - 1) * 512:] += res.results[core]["y3a"]
    y += np.asarray(b_out, dtype=np.float32)[None, None, :]
    return y



# revision 14
# speedup vs baseline: 1.0796x; 1.0796x over previous
"""Multi-head attention (QKV proj + rotary + softmax attention + out proj)
for Trainium2, sharded over 8 NeuronCores.

Problem: x[2,2048,1024], 16 heads x dh=64, rotary embedding, softmax
attention, output projection + bias.

Sharding: batch x head-group. Core c handles batch c//4 and the 4 heads
[4*(c%4), 4*(c%4)+4). Each core computes its QKV slice, rotary, attention,
and a partial output projection; the host sums the 4 partial projections
per batch and adds the bias.

Device-side design (per core, everything in "transposed" layout):
  - qkvT = W @ x^T computed as f32r matmuls (full PE rate, tf32-ish
    precision): qT/kT produced as [dh-pair(128), n] tiles, v as natural
    [n, e] tiles.
  - rotary applied on the fp32 psum output via DVE: q*cos +
    pairswap(q*sin_pre), with the dh dimension stored interleaved
    ([0,32,1,33,...]) so rotate_half becomes an adjacent-lane
    stream_shuffle. Output f32r.
  - dots: scoresT[j,n] = krotT^T-slice @ qrotT, two heads packed in the
    128x128 PE array via tile_position row-tiling (K=64 each). fp32 psum.
  - softmax without max-subtraction (logits are O(+-6)): ACT exp over
    2-j-tile psum batches (N=1024 per ACTIVATE), output fp16.
  - AV: lhsT = [v | ones] (M=65, fp16) so row 64 accumulates the softmax
    denominators for free; fp32 psum accumulation over the 16 j-tiles.
  - normalize: reciprocal_approx_fast of the sums row, partition-broadcast
    via a DRAM round-trip DMA (K=1 ones-matmul on the tail block), one DVE
    multiply straight out of psum -> aoT (f32r).
  - output proj: y[n,d] accumulated over the two head-pair e-chunks, f32r.

Perf structure (v1):
  - constants are loaded as per-128-column chunk tiles, emitted in
    need-order so the first matmul starts ~3us in instead of ~39us
    (whole-tile dependency tracking + ~22GB/s per DMA queue made the
    monolithic-tile preamble serialize).
  - y projections are split in halves and threaded into the pair-1
    attention blocks too, so the ACT-bound tail keeps the PE busy
    (HAM stays at K=8/8).
"""
import sys

sys.path.insert(0, "/opt/trn_rl_repo")

import numpy as np

import concourse.bacc as bacc
import concourse.tile as tile
from concourse import mybir
from concourse.bass_utils import run_bass_kernel_spmd

F32 = mybir.dt.float32
F32R = mybir.dt.float32r
BF16 = mybir.dt.bfloat16
FP16 = mybir.dt.float16
EXP = mybir.ActivationFunctionType.Exp
MULT = mybir.AluOpType.mult
ADD = mybir.AluOpType.add

B, N, DIM = 2, 2048, 1024
H, DH = 16, 64
INNER = H * DH
SCALE = DH ** -0.5
NCORES = 8
HPC = H // (NCORES // B)      # heads per core = 4
NPAIR = HPC // 2              # head pairs per core = 2

P = 128
NT = N // 512                 # 4 n-tiles of 512
DC = DIM // P                 # 8 d-chunks
JTILES = N // P               # 16 j-tiles
JB = JTILES // 2              # 8 j-batches (2 j-tiles each)

PAIRSWAP = [i ^ 1 for i in range(32)]

_CACHE = {}


def _build():
    nc = bacc.Bacc(None, target_bir_lowering=False, debug=False)
    with tile.TileContext(nc) as tc:
        with tc.tile_pool(name="dram", bufs=1, space="DRAM") as dram, \
             tc.tile_pool(name="const", bufs=1) as const, \
             tc.tile_pool(name="perst", bufs=1) as perst, \
             tc.tile_pool(name="tmp", bufs=1) as tmp, \
             tc.tile_pool(name="ps", bufs=1, space="PSUM") as ps:
            # ---------------- DRAM I/O ----------------
            xT_d = dram.tile([DIM, N], F32R, kind="ExternalInput", name="xT", uniquify=False)
            wqkT_d = dram.tile([DIM, 512], F32R, kind="ExternalInput", name="wqkT", uniquify=False)
            wvT_d = dram.tile([DIM, 256], F32R, kind="ExternalInput", name="wvT", uniquify=False)
            cq_d = dram.tile([P, N], F32, kind="ExternalInput", name="cq", uniquify=False)
            sq_d = dram.tile([P, N], F32, kind="ExternalInput", name="sq", uniquify=False)
            ck_d = dram.tile([P, N], F32, kind="ExternalInput", name="ck", uniquify=False)
            sk_d = dram.tile([P, N], F32, kind="ExternalInput", name="sk", uniquify=False)
            woT_d = dram.tile([256, DIM], F32R, kind="ExternalInput", name="woT", uniquify=False)
            y_d = dram.tile([N, DIM], F32, kind="ExternalOutput", name="y", uniquify=False)
            y3a_d = dram.tile([512, DIM], F32, kind="ExternalOutput", name="y3a", uniquify=False)

            xT_r = xT_d.rearrange("(c p) n -> p c n", p=P)
            wqk_r = wqkT_d.rearrange("(c p) e -> p c e", p=P)
            wv_r = wvT_d.rearrange("(c p) e -> p c e", p=P)
            wo_r = woT_d.rearrange("(c p) d -> p c d", p=P)

            # ------------- fine-grained const tiles -------------
            # e-chunk order in wqkT columns: q0=0, q1=1, k0=2, k1=3 (x128)
            wqk_sb = [[const.tile([P, P], F32R, name=f"wqk{e}_{c}")
                       for c in range(DC)] for e in range(4)]
            wv_sb = [const.tile([P, 256], F32R, name=f"wv{c}") for c in range(DC)]
            wo_sb = [[const.tile([P, 512], F32R, name=f"wo{p}_{d}")
                      for d in range(2)] for p in range(NPAIR)]
            cqt = [const.tile([P, 512], F32, name=f"cqt{t}") for t in range(NT)]
            sqt = [const.tile([P, 512], F32, name=f"sqt{t}") for t in range(NT)]
            ckt = [const.tile([P, 512], F32, name=f"ckt{t}") for t in range(NT)]
            skt = [const.tile([P, 512], F32, name=f"skt{t}") for t in range(NT)]

            def dma_wqk(e, c):
                nc.sync.dma_start(wqk_sb[e][c][:, :], wqk_r[:, c, e * P:(e + 1) * P])

            def dma_cs(tiles, src_d, t):
                nc.sync.dma_start(tiles[t][:, :], src_d[:, t * 512:(t + 1) * 512])

            # xt tiles for n-tile 0, loaded in the preamble
            xt0 = [tmp.tile([P, 512], F32R, name=f"xt{c}", tag=f"xt{c}", bufs=2)
                   for c in range(DC)]

            # ---------------- preamble emission (need-order) ----------------
            # round 1: k0 weights + x tile 0 spread over all 16 queues
            for c in range(DC):
                dma_wqk(2, c)
                nc.sync.dma_start(xt0[c][:, :], xT_r[:, c, 0:512])
            dma_cs(ckt, ck_d, 0)
            dma_cs(skt, sk_d, 0)
            for c in range(DC):
                dma_wqk(0, c)          # q0
            dma_cs(cqt, cq_d, 0)
            dma_cs(sqt, sq_d, 0)
            for c in range(DC):
                nc.sync.dma_start(wv_sb[c][:, :], wv_r[:, c, :])
            for c in range(DC):
                dma_wqk(3, c)          # k1
            for c in range(DC):
                dma_wqk(1, c)          # q1
            for t in range(1, NT):
                dma_cs(ckt, ck_d, t)
                dma_cs(skt, sk_d, t)
            for t in range(1, NT):
                dma_cs(cqt, cq_d, t)
                dma_cs(sqt, sq_d, t)
            for p in range(NPAIR):
                for d in range(2):
                    nc.sync.dma_start(wo_sb[p][d][:, :], wo_r[:, p, d * 512:(d + 1) * 512])

            ones_f = const.tile([1, 64], F32)
            nc.vector.memset(ones_f[:, :], 1.0)
            ones_r = const.tile([1, 64], F32R)
            nc.vector.tensor_copy(ones_r[:, :], ones_f[:, :])

            # ---------------- persistent tiles ----------------
            qrot = [[perst.tile([P, 512], F32R, name=f"qrot{p}_{t}")
                     for t in range(NT)] for p in range(NPAIR)]
            krot = [[perst.tile([P, 512], F32R, name=f"krot{p}_{t}")
                     for t in range(NT)] for p in range(NPAIR)]
            # AV stationary operand is [ones | zeros(63) | v(64)] so psum row 0
            # accumulates the softmax denominators (base_partition 0 for the
            # DVE reciprocal) and the values land at rows 64-127 (32-aligned
            # partition base) -- one aligned evacuation copy serves both.
            v_aug = [perst.tile([P, 4, HPC, P], FP16, name=f"vaug{t}")
                     for t in range(NT)]
            for t in range(NT):
                nc.vector.memset(v_aug[t][:, :, :, 0:64], 0.0)
                nc.vector.memset(v_aug[t][:, :, :, 0:1], 1.0)
            aoT = [[perst.tile([P, 512], F32R, name=f"aoT{p}_{t}")
                    for t in range(NT)] for p in range(NPAIR)]

            # ---------------- helpers ----------------
            def load_x(t):
                # per-d-chunk tiles so matmuls can start as soon as the first
                # 256KB chunk lands instead of waiting for the full 2MB tile
                xt = [tmp.tile([P, 512], F32R, name=f"xt{c}", tag=f"xt{c}", bufs=2)
                      for c in range(DC)]
                for c in range(DC):
                    nc.sync.dma_start(xt[c][:, :], xT_r[:, c, t * 512:(t + 1) * 512])
                return xt

            def qk_chunk(ech, t, xt, dest, cos_t, sin_t):
                # qkvT e-chunk [128, 512] = W-chunk @ xT-tile, then rotary.
                pqk = ps.tile([P, 512], F32, name="pqk", tag="m", bufs=2)
                for c in range(DC):
                    nc.tensor.matmul(pqk[:, :],
                                     wqk_sb[ech][c][:, :],
                                     xt[c][:, :],
                                     start=(c == 0), stop=(c == DC - 1))
                t1 = tmp.tile([P, 512], F32, name="t1", tag="t1", bufs=2)
                t2 = tmp.tile([P, 512], F32, name="t2", tag="t2", bufs=2)
                t3 = tmp.tile([P, 512], F32, name="t3", tag="t3", bufs=2)
                nc.vector.tensor_tensor(t1[:, :], pqk[:, :], cos_t[t][:, :], op=MULT)
                nc.vector.tensor_tensor(t2[:, :], pqk[:, :], sin_t[t][:, :], op=MULT)
                nc.vector.stream_shuffle(t3[:, :], t2[:, :], PAIRSWAP)
                nc.vector.tensor_tensor(dest[:, :], t1[:, :], t3[:, :], op=ADD)

            def v_tile(t, xt):
                # v natural [n, e] for the 4 local heads, by 128-row subtiles
                for nsub in range(4):
                    pv = ps.tile([P, 256], F32, name="pv", tag="m", bufs=2)
                    for c in range(DC):
                        nc.tensor.matmul(pv[:, :],
                                         xt[c][:, nsub * P:(nsub + 1) * P],
                                         wv_sb[c][:, :],
                                         start=(c == 0), stop=(c == DC - 1))
                    nc.vector.tensor_copy(
                        v_aug[t][:, nsub, :, 64:128],
                        pv[:, :].rearrange("p (h d) -> p h d", h=HPC))

            def qkv_for_tile(t, ops, xt=None):
                if xt is None:
                    xt = load_x(t)
                for op in ops:
                    if op == "k0":
                        qk_chunk(2, t, xt, krot[0][t], ckt, skt)
                    elif op == "k1":
                        qk_chunk(3, t, xt, krot[1][t], ckt, skt)
                    elif op == "q0":
                        qk_chunk(0, t, xt, qrot[0][t], cqt, sqt)
                    elif op == "q1":
                        qk_chunk(1, t, xt, qrot[1][t], cqt, sqt)
                    elif op == "v":
                        v_tile(t, xt)

            def attention(nq, pair, pre_jb=None, mid_jb=None):
                pav = [ps.tile([P, 512], F32, name=f"pav{h}", tag="av", bufs=2)
                       for h in range(2)]
                for jb in range(JB):
                    if pre_jb is not None:
                        pre_jb(jb)
                    sc = [ps.tile([P, 2, 512], F32, name=f"sc{h}", tag="s", bufs=2)
                          for h in range(2)]
                    for jl in range(2):
                        jt = jb * 2 + jl
                        kt = krot[pair][jt // 4]
                        jsl = slice((jt % 4) * P, (jt % 4 + 1) * P)
                        for h in range(2):
                            rows = slice(h * 64, (h + 1) * 64)
                            nc.tensor.matmul(sc[h][:, jl, :],
                                             kt[rows, jsl],
                                             qrot[pair][nq][rows, :],
                                             start=True, stop=True,
                                             tile_position=(h * 64, 0))
                    ex = [tmp.tile([P, 2, 512], FP16, name=f"ex{h}", tag="ex", bufs=4)
                          for h in range(2)]
                    for h in range(2):
                        nc.scalar.activation(ex[h][:, :, :], sc[h][:, :, :], EXP)
                    if mid_jb is not None:
                        mid_jb(jb)
                    for jl in range(2):
                        jt = jb * 2 + jl
                        for h in range(2):
                            nc.tensor.matmul(pav[h][:, :],
                                             v_aug[jt // 4][:, jt % 4, pair * 2 + h, :],
                                             ex[h][:, jl, :],
                                             start=(jt == 0), stop=(jt == JTILES - 1))
                for h in range(2):
                    # evacuate psum immediately (one copy: row 0 = denominators
                    # at base_partition 0 as the custom-DVE reciprocal needs,
                    # rows 64-127 = AV values) so the next block's AV can start
                    av_sb = tmp.tile([P, 512], F32, name="av_sb", tag="avs", bufs=2)
                    nc.vector.tensor_copy(av_sb[:, :], pav[h][:, :])
                    rc = tmp.tile([1, 512], F32, name="rc", tag="rc", bufs=2)
                    nc.vector.reciprocal_approx_fast(rc[:, :], av_sb[0:1, :])
                    bc = tmp.tile([P, 512], F32, name="bc", tag="bc", bufs=2)
                    if nq == NT - 1:
                        # tail-critical: broadcast via K=1 ones-matmul (no DMA
                        # round-trip latency before the last y projection)
                        rcr = tmp.tile([1, 512], F32R, name="rcr", tag="rcr", bufs=2)
                        nc.vector.tensor_copy(rcr[:, :], rc[:, :])
                        pbc = ps.tile([64, 512], F32, name="pbc", tag="m", bufs=2)
                        nc.tensor.matmul(pbc[:, :], ones_r[:, :], rcr[:, :],
                                         start=True, stop=True)
                        nc.vector.tensor_copy(bc[64:128, :], pbc[:, :])
                    else:
                        # broadcast across partitions via a DRAM round-trip
                        rd = dram.tile([1, 512], F32, name="rd", tag="rd", bufs=2)
                        nc.sync.dma_start(rd[:, :], rc[:, :])
                        nc.sync.dma_start(bc[64:128, :], rd.to_broadcast([64, 512]))
                    rows = slice(h * 64, (h + 1) * 64)
                    nc.vector.tensor_tensor(aoT[pair][nq][rows, :],
                                            av_sb[64:128, :], bc[64:128, :], op=MULT)

            def y_proj_nsub(nq, nsub, out_d, row0):
                # both-pair projection for one 128-row query subtile
                ys = tmp.tile([P, DIM], F32, name="ys", tag="ys", bufs=2)
                nsl = slice(nsub * P, (nsub + 1) * P)
                for dh2 in range(2):
                    py = ps.tile([P, 512], F32, name="py", tag="m", bufs=2)
                    dsl = slice(dh2 * 512, (dh2 + 1) * 512)
                    for pair in range(NPAIR):
                        nc.tensor.matmul(py[:, :],
                                         aoT[pair][nq][:, nsl],
                                         wo_sb[pair][dh2][:, :],
                                         start=(pair == 0), stop=(pair == NPAIR - 1))
                    nc.vector.tensor_copy(ys[:, dsl], py[:, :])
                nc.sync.dma_start(out_d[row0 + nsub * P:row0 + (nsub + 1) * P, :],
                                  ys[:, :])

            def y_proj_half(nq, half):
                for nsub in ((0, 1) if half == 0 else (2, 3)):
                    y_proj_nsub(nq, nsub, y_d, nq * 512)

            def y_proj_pair(nq, pair, out_d, row0):
                # single-pair partial projection (no cross-pair accumulation)
                for nsub in range(4):
                    ys = tmp.tile([P, DIM], F32, name="ysp", tag="ys", bufs=2)
                    nsl = slice(nsub * P, (nsub + 1) * P)
                    for dh2 in range(2):
                        py = ps.tile([P, 512], F32, name="pyp", tag="m", bufs=2)
                        dsl = slice(dh2 * 512, (dh2 + 1) * 512)
                        nc.tensor.matmul(py[:, :], aoT[pair][nq][:, nsl],
                                         wo_sb[pair][dh2][:, :],
                                         start=True, stop=True)
                        nc.vector.tensor_copy(ys[:, dsl], py[:, :])
                    nc.sync.dma_start(out_d[row0 + nsub * P:row0 + (nsub + 1) * P, :],
                                      ys[:, :])

            # ---------------- emission order ----------------
            # Tile has sequential program-order semantics: every tile must be
            # written (in emission order) before anything that reads it, and
            # per-psum-tag slot reuse is FIFO in emission order. QKV work and
            # the output projections are threaded just-in-time through the
            # attention j-loops: k before the dots that need it, v between exp
            # and the AV that needs it, next-q early, y-projection halves into
            # BOTH pair blocks so the ACT-bound stretches keep the PE fed.
            qkv_for_tile(0, ["k0", "q0"], xt=xt0)
            cur_xt = {0: xt0}

            def pre_first(jb):
                if jb == 1:
                    qkv_for_tile(0, ["k1", "q1"], xt=cur_xt.pop(0))
                elif jb in (2, 4, 6):
                    t = jb // 2
                    cur_xt[t] = load_x(t)
                    qkv_for_tile(t, ["k0", "k1"], xt=cur_xt[t])

            def mid_first(jb):
                if jb in (0, 2, 4, 6):
                    t = jb // 2
                    if t == 0:
                        qkv_for_tile(0, ["v"], xt=xt0)
                    else:
                        qkv_for_tile(t, ["v"], xt=cur_xt.pop(t))

            def make_pre(nq, pair):
                def pre(jb):
                    if jb == 1 and pair == 0 and nq + 1 < NT:
                        qkv_for_tile(nq + 1, ["q0", "q1"])
                    if jb == 4 and nq >= 1:
                        y_proj_half(nq - 1, pair)
                    if jb == 2 and (nq, pair) == (NT - 1, 1):
                        y_proj_pair(NT - 1, 0, y3a_d, 0)
                return pre

            for nq in range(NT):
                for pair in range(NPAIR):
                    if nq == 0 and pair == 0:
                        attention(nq, pair, pre_jb=pre_first, mid_jb=mid_first)
                    elif nq == 0 and pair == 1:
                        def pre01(jb):
                            if jb == 1:
                                qkv_for_tile(1, ["q0", "q1"])
                        attention(nq, pair, pre_jb=pre01)
                    else:
                        attention(nq, pair, pre_jb=make_pre(nq, pair))
            y_proj_pair(NT - 1, 1, y_d, (NT - 1) * 512)
    nc.compile()
    return nc


def _host_prep(x, rotary_emb, w_qkv, w_out):
    """Build the 8 per-core input maps."""
    x = np.asarray(x, dtype=np.float32)
    rotary_emb = np.asarray(rotary_emb, dtype=np.float32)
    w_qkv = np.asarray(w_qkv, dtype=np.float32)
    w_out = np.asarray(w_out, dtype=np.float32)

    # interleaved dh permutation: new row 2i <- dim i, 2i+1 <- dim 32+i
    perm = np.empty(DH, dtype=np.int64)
    perm[0::2] = np.arange(32)
    perm[1::2] = np.arange(32) + 32
    pair_swap = np.arange(DH) ^ 1

    cos = np.cos(rotary_emb).T[perm]                      # [dh, n] permuted
    sin = np.sin(rotary_emb).T[perm]
    sign = np.where(perm < 32, -1.0, 1.0)[:, None].astype(np.float32)
    sin_eff = sign * sin
    sin_pre = sin_eff[pair_swap]                          # pre-swapped
    c2 = np.concatenate([cos, cos], axis=0)               # [128, n]
    s2 = np.concatenate([sin_pre, sin_pre], axis=0)
    cq = np.ascontiguousarray(SCALE * c2)
    sq = np.ascontiguousarray(SCALE * s2)
    ck = np.ascontiguousarray(c2)
    sk = np.ascontiguousarray(s2)

    in_maps = []
    for core in range(NCORES):
        b = core // (NCORES // B)
        g = core % (NCORES // B)
        heads = range(4 * g, 4 * g + HPC)
        q_rows = np.concatenate([h * DH + perm for h in heads])
        k_rows = np.concatenate([INNER + h * DH + perm for h in heads])
        v_rows = np.arange(2 * INNER + 4 * g * DH, 2 * INNER + (4 * g + HPC) * DH)
        wqkT = np.ascontiguousarray(w_qkv[np.concatenate([q_rows, k_rows])].T)
        wvT = np.ascontiguousarray(w_qkv[v_rows].T)
        woT = np.ascontiguousarray(w_out[:, 4 * g * DH:(4 * g + HPC) * DH].T)
        xT = np.ascontiguousarray(x[b].T)
        in_maps.append({
            "xT": xT, "wqkT": wqkT, "wvT": wvT,
            "cq": cq, "sq": sq, "ck": ck, "sk": sk, "woT": woT,
        })
    return in_maps


def kernel(x, rotary_emb, w_qkv, w_out, b_out, _trace=False):
    if "nc" not in _CACHE:
        _CACHE["nc"] = _build()
    nc = _CACHE["nc"]
    in_maps = _host_prep(x, rotary_emb, w_qkv, w_out)
    res = run_bass_kernel_spmd(nc, in_maps, core_ids=list(range(NCORES)),
                               trace=_trace)
    _CACHE["last_result"] = res
    y = np.zeros((B, N, DIM), dtype=np.float32)
    for core in range(NCORES):
        b = core // (NCORES // B)
        y[b] += res.results[core]["y"]
        y[b, (NT - 1) * 512:] += res.results[core]["y3a"]
    y += np.asarray(b_out, dtype=np.float32)[None, None, :]
    return y


# revision 21
# speedup vs baseline: 1.2248x; 1.1344x over previous
"""Multi-head attention (QKV proj + rotary + softmax attention + out proj)
for Trainium2, sharded over 8 NeuronCores.

Problem: x[2,2048,1024], 16 heads x dh=64, rotary embedding, softmax
attention, output projection + bias.

Sharding: batch x head-group. Core c handles batch c//4 and the 4 heads
[4*(c%4), 4*(c%4)+4). Each core computes its QKV slice, rotary, attention,
and a partial output projection; the host sums the 4 partial projections
per batch and adds the bias.

Device-side design (per core, everything in "transposed" layout):
  - qkvT = W @ x^T computed as f32r matmuls (full PE rate, tf32-ish
    precision): qT/kT produced as [dh-pair(128), n] tiles, v as natural
    [n, e] tiles.
  - rotary applied on the fp32 psum output via DVE: q*cos +
    pairswap(q*sin_pre), with the dh dimension stored interleaved
    ([0,32,1,33,...]) so rotate_half becomes an adjacent-lane
    stream_shuffle. Output f32r.
  - dots: scoresT[j,n] = krotT^T-slice @ qrotT, two heads packed in the
    128x128 PE array via tile_position row-tiling (K=64 each). fp32 psum.
  - softmax without max-subtraction (logits are O(+-6)): ACT exp over
    2-j-tile psum batches (N=1024 per ACTIVATE), output fp16.
  - AV: lhsT = [v | ones] (M=65, fp16) so row 64 accumulates the softmax
    denominators for free; fp32 psum accumulation over the 16 j-tiles.
  - normalize: reciprocal_approx_fast of the sums row, partition-broadcast
    via a DRAM round-trip DMA (K=1 ones-matmul on the tail block), one DVE
    multiply straight out of psum -> aoT (f32r).
  - output proj: y[n,d] accumulated over the two head-pair e-chunks, f32r.

Perf structure (v1):
  - constants are loaded as per-128-column chunk tiles, emitted in
    need-order so the first matmul starts ~3us in instead of ~39us
    (whole-tile dependency tracking + ~22GB/s per DMA queue made the
    monolithic-tile preamble serialize).
  - y projections are split in halves and threaded into the pair-1
    attention blocks too, so the ACT-bound tail keeps the PE busy
    (HAM stays at K=8/8).
"""
import sys

sys.path.insert(0, "/opt/trn_rl_repo")

import numpy as np

import concourse.bacc as bacc
import concourse.tile as tile
from concourse import mybir
from concourse.bass_utils import run_bass_kernel_spmd

F32 = mybir.dt.float32
F32R = mybir.dt.float32r
BF16 = mybir.dt.bfloat16
FP16 = mybir.dt.float16
EXP = mybir.ActivationFunctionType.Exp
MULT = mybir.AluOpType.mult
ADD = mybir.AluOpType.add

B, N, DIM = 2, 2048, 1024
H, DH = 16, 64
INNER = H * DH
SCALE = DH ** -0.5
NCORES = 8
HPC = H // (NCORES // B)      # heads per core = 4
NPAIR = HPC // 2              # head pairs per core = 2

P = 128
NT = N // 512                 # 4 n-tiles of 512
DC = DIM // P                 # 8 d-chunks
JTILES = N // P               # 16 j-tiles
JB = JTILES // 2              # 8 j-batches (2 j-tiles each)

PAIRSWAP = [i ^ 1 for i in range(32)]

_CACHE = {}


def _build():
    nc = bacc.Bacc(None, target_bir_lowering=False, debug=False)
    with tile.TileContext(nc) as tc:
        with tc.tile_pool(name="dram", bufs=1, space="DRAM") as dram, \
             tc.tile_pool(name="const", bufs=1) as const, \
             tc.tile_pool(name="perst", bufs=1) as perst, \
             tc.tile_pool(name="tmp", bufs=1) as tmp, \
             tc.tile_pool(name="ps", bufs=1, space="PSUM") as ps:
            # ---------------- DRAM I/O ----------------
            xT_d = dram.tile([DIM, N], BF16, kind="ExternalInput", name="xT", uniquify=False)
            wqkT_d = dram.tile([DIM, 512], BF16, kind="ExternalInput", name="wqkT", uniquify=False)
            wvT_d = dram.tile([DIM, 256], BF16, kind="ExternalInput", name="wvT", uniquify=False)
            cq_d = dram.tile([P, N], BF16, kind="ExternalInput", name="cq", uniquify=False)
            sq_d = dram.tile([P, N], BF16, kind="ExternalInput", name="sq", uniquify=False)
            ck_d = dram.tile([P, N], BF16, kind="ExternalInput", name="ck", uniquify=False)
            sk_d = dram.tile([P, N], BF16, kind="ExternalInput", name="sk", uniquify=False)
            woT_d = dram.tile([256, DIM], BF16, kind="ExternalInput", name="woT", uniquify=False)
            y_d = dram.tile([N, DIM], F32, kind="ExternalOutput", name="y", uniquify=False)
            y3a_d = dram.tile([512, DIM], F32, kind="ExternalOutput", name="y3a", uniquify=False)

            xT_r = xT_d.rearrange("(c p) n -> p c n", p=P)
            wqk_r = wqkT_d.rearrange("(c p) e -> p c e", p=P)
            wv_r = wvT_d.rearrange("(c p) e -> p c e", p=P)
            wo_r = woT_d.rearrange("(c p) d -> p c d", p=P)

            # ------------- fine-grained const tiles -------------
            # e-chunk order in wqkT columns: q0=0, q1=1, k0=2, k1=3 (x128)
            wqk_sb = [[const.tile([P, P], BF16, name=f"wqk{e}_{c}")
                       for c in range(DC)] for e in range(4)]
            wv_sb = [const.tile([P, 256], BF16, name=f"wv{c}") for c in range(DC)]
            wo_sb = [[const.tile([P, 512], BF16, name=f"wo{p}_{d}")
                      for d in range(2)] for p in range(NPAIR)]
            cqt = [const.tile([P, 512], BF16, name=f"cqt{t}") for t in range(NT)]
            sqt = [const.tile([P, 512], BF16, name=f"sqt{t}") for t in range(NT)]
            ckt = [const.tile([P, 512], BF16, name=f"ckt{t}") for t in range(NT)]
            skt = [const.tile([P, 512], BF16, name=f"skt{t}") for t in range(NT)]

            def dma_wqk(e, c):
                nc.sync.dma_start(wqk_sb[e][c][:, :], wqk_r[:, c, e * P:(e + 1) * P])

            def dma_cs(tiles, src_d, t):
                nc.sync.dma_start(tiles[t][:, :], src_d[:, t * 512:(t + 1) * 512])

            # all x tiles prefetched in the preamble (bf16: 4MB total) and
            # kept resident for the whole kernel -- x is loaded exactly once
            xt_all = [[tmp.tile([P, 512], BF16, name=f"xt{t}_{c}",
                                tag=f"xt{c}", bufs=NT) for c in range(DC)]
                      for t in range(NT)]

            def dma_x(t):
                for c in range(DC):
                    nc.sync.dma_start(xt_all[t][c][:, :],
                                      xT_r[:, c, t * 512:(t + 1) * 512])

            # ---------------- preamble emission (need-order) ----------------
            # round 1: k0 weights + x tile 0 spread over all 16 queues
            for c in range(DC):
                dma_wqk(2, c)
                nc.sync.dma_start(xt_all[0][c][:, :], xT_r[:, c, 0:512])
            dma_cs(ckt, ck_d, 0)
            dma_cs(skt, sk_d, 0)
            for c in range(DC):
                dma_wqk(0, c)          # q0
            dma_cs(cqt, cq_d, 0)
            dma_cs(sqt, sq_d, 0)
            for c in range(DC):
                nc.sync.dma_start(wv_sb[c][:, :], wv_r[:, c, :])
            dma_x(1)
            for c in range(DC):
                dma_wqk(3, c)          # k1
            for c in range(DC):
                dma_wqk(1, c)          # q1
            dma_x(2)
            for t in range(1, NT):
                dma_cs(ckt, ck_d, t)
                dma_cs(skt, sk_d, t)
            dma_x(3)
            for t in range(1, NT):
                dma_cs(cqt, cq_d, t)
                dma_cs(sqt, sq_d, t)
            for p in range(NPAIR):
                for d in range(2):
                    nc.sync.dma_start(wo_sb[p][d][:, :], wo_r[:, p, d * 512:(d + 1) * 512])

            ones_f = const.tile([1, 64], F32)
            nc.vector.memset(ones_f[:, :], 1.0)
            ones_r = const.tile([1, 64], F32R)
            nc.vector.tensor_copy(ones_r[:, :], ones_f[:, :])

            # ---------------- persistent tiles ----------------
            qrot = [[perst.tile([P, 512], BF16, name=f"qrot{p}_{t}")
                     for t in range(NT)] for p in range(NPAIR)]
            krot = [[perst.tile([P, 512], BF16, name=f"krot{p}_{t}")
                     for t in range(NT)] for p in range(NPAIR)]
            # AV stationary operand is [ones | zeros(63) | v(64)] so psum row 0
            # accumulates the softmax denominators (base_partition 0 for the
            # DVE reciprocal) and the values land at rows 64-127 (32-aligned
            # partition base) -- one aligned evacuation copy serves both.
            v_aug = [perst.tile([P, 4, HPC, P], FP16, name=f"vaug{t}")
                     for t in range(NT)]
            for t in range(NT):
                nc.vector.memset(v_aug[t][:, :, :, 0:64], 0.0)
                nc.vector.memset(v_aug[t][:, :, :, 0:1], 1.0)
            aoT = [[perst.tile([P, 512], BF16, name=f"aoT{p}_{t}")
                    for t in range(NT)] for p in range(NPAIR)]

            # ---------------- helpers ----------------
            def qk_chunk(ech, t, xt, dest, cos_t, sin_t):
                # qkvT e-chunk [128, 512] = W-chunk @ xT-tile, then rotary.
                pqk = ps.tile([P, 512], F32, name="pqk", tag="m", bufs=2)
                for c in range(DC):
                    nc.tensor.matmul(pqk[:, :],
                                     wqk_sb[ech][c][:, :],
                                     xt[c][:, :],
                                     start=(c == 0), stop=(c == DC - 1))
                t1 = tmp.tile([P, 512], BF16, name="t1", tag="t1", bufs=2)
                t2 = tmp.tile([P, 512], BF16, name="t2", tag="t2", bufs=2)
                t3 = tmp.tile([P, 512], BF16, name="t3", tag="t3", bufs=2)
                nc.vector.tensor_tensor(t1[:, :], pqk[:, :], cos_t[t][:, :], op=MULT)
                nc.vector.tensor_tensor(t2[:, :], pqk[:, :], sin_t[t][:, :], op=MULT)
                nc.vector.stream_shuffle(t3[:, :], t2[:, :], PAIRSWAP)
                nc.vector.tensor_tensor(dest[:, :], t1[:, :], t3[:, :], op=ADD)

            def v_tile(t, xt):
                # v natural [n, e] for the 4 local heads, by 128-row subtiles
                for nsub in range(4):
                    pv = ps.tile([P, 256], F32, name="pv", tag="m", bufs=2)
                    for c in range(DC):
                        nc.tensor.matmul(pv[:, :],
                                         xt[c][:, nsub * P:(nsub + 1) * P],
                                         wv_sb[c][:, :],
                                         start=(c == 0), stop=(c == DC - 1))
                    nc.vector.tensor_copy(
                        v_aug[t][:, nsub, :, 64:128],
                        pv[:, :].rearrange("p (h d) -> p h d", h=HPC))

            def qkv_for_tile(t, ops):
                xt = xt_all[t]
                for op in ops:
                    if op == "k0":
                        qk_chunk(2, t, xt, krot[0][t], ckt, skt)
                    elif op == "k1":
                        qk_chunk(3, t, xt, krot[1][t], ckt, skt)
                    elif op == "q0":
                        qk_chunk(0, t, xt, qrot[0][t], cqt, sqt)
                    elif op == "q1":
                        qk_chunk(1, t, xt, qrot[1][t], cqt, sqt)
                    elif op == "v":
                        v_tile(t, xt)

            def attention(nq, pair, pre_jb=None, mid_jb=None):
                pav = [ps.tile([P, 512], F32, name=f"pav{h}", tag="av", bufs=2)
                       for h in range(2)]
                for jb in range(JB):
                    if pre_jb is not None:
                        pre_jb(jb)
                    sc = [ps.tile([P, 2, 512], F32, name=f"sc{h}", tag="s", bufs=2)
                          for h in range(2)]
                    for jl in range(2):
                        jt = jb * 2 + jl
                        kt = krot[pair][jt // 4]
                        jsl = slice((jt % 4) * P, (jt % 4 + 1) * P)
                        for h in range(2):
                            rows = slice(h * 64, (h + 1) * 64)
                            nc.tensor.matmul(sc[h][:, jl, :],
                                             kt[rows, jsl],
                                             qrot[pair][nq][rows, :],
                                             start=True, stop=True,
                                             tile_position=(h * 64, 0))
                    ex = [tmp.tile([P, 2, 512], FP16, name=f"ex{h}", tag="ex", bufs=4)
                          for h in range(2)]
                    for h in range(2):
                        nc.scalar.activation(ex[h][:, :, :], sc[h][:, :, :], EXP)
                    if mid_jb is not None:
                        mid_jb(jb)
                    for jl in range(2):
                        jt = jb * 2 + jl
                        for h in range(2):
                            nc.tensor.matmul(pav[h][:, :],
                                             v_aug[jt // 4][:, jt % 4, pair * 2 + h, :],
                                             ex[h][:, jl, :],
                                             start=(jt == 0), stop=(jt == JTILES - 1))
                for h in range(2):
                    # evacuate psum immediately (one copy: row 0 = denominators
                    # at base_partition 0 as the custom-DVE reciprocal needs,
                    # rows 64-127 = AV values) so the next block's AV can start
                    av_sb = tmp.tile([P, 512], F32, name="av_sb", tag="avs", bufs=2)
                    nc.vector.tensor_copy(av_sb[:, :], pav[h][:, :])
                    rc = tmp.tile([1, 512], F32, name="rc", tag="rc", bufs=2)
                    nc.vector.reciprocal_approx_fast(rc[:, :], av_sb[0:1, :])
                    bc = tmp.tile([P, 512], F32, name="bc", tag="bc", bufs=2)
                    if nq == NT - 1:
                        # tail-critical: broadcast via K=1 ones-matmul (no DMA
                        # round-trip latency before the last y projection)
                        rcr = tmp.tile([1, 512], F32R, name="rcr", tag="rcr", bufs=2)
                        nc.vector.tensor_copy(rcr[:, :], rc[:, :])
                        pbc = ps.tile([64, 512], F32, name="pbc", tag="m", bufs=2)
                        nc.tensor.matmul(pbc[:, :], ones_r[:, :], rcr[:, :],
                                         start=True, stop=True)
                        nc.vector.tensor_copy(bc[64:128, :], pbc[:, :])
                    else:
                        # broadcast across partitions via a DRAM round-trip
                        rd = dram.tile([1, 512], F32, name="rd", tag="rd", bufs=2)
                        nc.sync.dma_start(rd[:, :], rc[:, :])
                        nc.sync.dma_start(bc[64:128, :], rd.to_broadcast([64, 512]))
                    rows = slice(h * 64, (h + 1) * 64)
                    nc.vector.tensor_tensor(aoT[pair][nq][rows, :],
                                            av_sb[64:128, :], bc[64:128, :], op=MULT)

            def y_proj_nsub(nq, nsub, out_d, row0):
                # both-pair projection for one 128-row query subtile
                ys = tmp.tile([P, DIM], F32, name="ys", tag="ys", bufs=2)
                nsl = slice(nsub * P, (nsub + 1) * P)
                for dh2 in range(2):
                    py = ps.tile([P, 512], F32, name="py", tag="m", bufs=2)
                    dsl = slice(dh2 * 512, (dh2 + 1) * 512)
                    for pair in range(NPAIR):
                        nc.tensor.matmul(py[:, :],
                                         aoT[pair][nq][:, nsl],
                                         wo_sb[pair][dh2][:, :],
                                         start=(pair == 0), stop=(pair == NPAIR - 1))
                    nc.vector.tensor_copy(ys[:, dsl], py[:, :])
                nc.sync.dma_start(out_d[row0 + nsub * P:row0 + (nsub + 1) * P, :],
                                  ys[:, :])

            def y_proj_half(nq, half):
                for nsub in ((0, 1) if half == 0 else (2, 3)):
                    y_proj_nsub(nq, nsub, y_d, nq * 512)

            def y_proj_pair(nq, pair, out_d, row0):
                # single-pair partial projection (no cross-pair accumulation)
                for nsub in range(4):
                    ys = tmp.tile([P, DIM], F32, name="ysp", tag="ys", bufs=2)
                    nsl = slice(nsub * P, (nsub + 1) * P)
                    for dh2 in range(2):
                        py = ps.tile([P, 512], F32, name="pyp", tag="m", bufs=2)
                        dsl = slice(dh2 * 512, (dh2 + 1) * 512)
                        nc.tensor.matmul(py[:, :], aoT[pair][nq][:, nsl],
                                         wo_sb[pair][dh2][:, :],
                                         start=True, stop=True)
                        nc.vector.tensor_copy(ys[:, dsl], py[:, :])
                    nc.sync.dma_start(out_d[row0 + nsub * P:row0 + (nsub + 1) * P, :],
                                      ys[:, :])

            # ---------------- emission order ----------------
            # Tile has sequential program-order semantics: every tile must be
            # written (in emission order) before anything that reads it, and
            # per-psum-tag slot reuse is FIFO in emission order. QKV work and
            # the output projections are threaded just-in-time through the
            # attention j-loops: k before the dots that need it, v between exp
            # and the AV that needs it, next-q early, y-projection halves into
            # BOTH pair blocks so the ACT-bound stretches keep the PE fed.
            qkv_for_tile(0, ["k0", "q0"])

            def pre_first(jb):
                if jb == 1:
                    qkv_for_tile(0, ["k1", "q1"])
                elif jb in (2, 4, 6):
                    qkv_for_tile(jb // 2, ["k0", "k1"])

            def mid_first(jb):
                if jb in (0, 2, 4, 6):
                    qkv_for_tile(jb // 2, ["v"])

            def make_pre(nq, pair):
                def pre(jb):
                    if jb == 1 and pair == 0 and nq + 1 < NT:
                        qkv_for_tile(nq + 1, ["q0", "q1"])
                    if jb == 4 and nq >= 1:
                        y_proj_half(nq - 1, pair)
                    if jb == 2 and (nq, pair) == (NT - 1, 1):
                        y_proj_pair(NT - 1, 0, y3a_d, 0)
                return pre

            for nq in range(NT):
                for pair in range(NPAIR):
                    if nq == 0 and pair == 0:
                        attention(nq, pair, pre_jb=pre_first, mid_jb=mid_first)
                    elif nq == 0 and pair == 1:
                        def pre01(jb):
                            if jb == 1:
                                qkv_for_tile(1, ["q0", "q1"])
                        attention(nq, pair, pre_jb=pre01)
                    else:
                        attention(nq, pair, pre_jb=make_pre(nq, pair))
            y_proj_pair(NT - 1, 1, y_d, (NT - 1) * 512)
    nc.compile()
    return nc


def _host_prep(x, rotary_emb, w_qkv, w_out):
    """Build the 8 per-core input maps."""
    x = np.asarray(x, dtype=np.float32)
    rotary_emb = np.asarray(rotary_emb, dtype=np.float32)
    w_qkv = np.asarray(w_qkv, dtype=np.float32)
    w_out = np.asarray(w_out, dtype=np.float32)

    # interleaved dh permutation: new row 2i <- dim i, 2i+1 <- dim 32+i
    perm = np.empty(DH, dtype=np.int64)
    perm[0::2] = np.arange(32)
    perm[1::2] = np.arange(32) + 32
    pair_swap = np.arange(DH) ^ 1

    import ml_dtypes
    bf16 = ml_dtypes.bfloat16

    cos = np.cos(rotary_emb).T[perm]                      # [dh, n] permuted
    sin = np.sin(rotary_emb).T[perm]
    sign = np.where(perm < 32, -1.0, 1.0)[:, None].astype(np.float32)
    sin_eff = sign * sin
    sin_pre = sin_eff[pair_swap]                          # pre-swapped
    c2 = np.concatenate([cos, cos], axis=0)               # [128, n]
    s2 = np.concatenate([sin_pre, sin_pre], axis=0)
    cq = np.ascontiguousarray((SCALE * c2).astype(bf16))
    sq = np.ascontiguousarray((SCALE * s2).astype(bf16))
    ck = np.ascontiguousarray(c2.astype(bf16))
    sk = np.ascontiguousarray(s2.astype(bf16))

    in_maps = []
    for core in range(NCORES):
        b = core // (NCORES // B)
        g = core % (NCORES // B)
        heads = range(4 * g, 4 * g + HPC)
        q_rows = np.concatenate([h * DH + perm for h in heads])
        k_rows = np.concatenate([INNER + h * DH + perm for h in heads])
        v_rows = np.arange(2 * INNER + 4 * g * DH, 2 * INNER + (4 * g + HPC) * DH)
        wqkT = np.ascontiguousarray(w_qkv[np.concatenate([q_rows, k_rows])].T.astype(bf16))
        wvT = np.ascontiguousarray(w_qkv[v_rows].T.astype(bf16))
        woT = np.ascontiguousarray(w_out[:, 4 * g * DH:(4 * g + HPC) * DH].T.astype(bf16))
        xT = np.ascontiguousarray(x[b].T.astype(bf16))
        in_maps.append({
            "xT": xT, "wqkT": wqkT, "wvT": wvT,
            "cq": cq, "sq": sq, "ck": ck, "sk": sk, "woT": woT,
        })
    return in_maps


def kernel(x, rotary_emb, w_qkv, w_out, b_out, _trace=False):
    if "nc" not in _CACHE:
        _CACHE["nc"] = _build()
    nc = _CACHE["nc"]
    in_maps = _host_prep(x, rotary_emb, w_qkv, w_out)
    res = run_bass_kernel_spmd(nc, in_maps, core_ids=list(range(NCORES)),
                               trace=_trace)
    _CACHE["last_result"] = res
    y = np.zeros((B, N, DIM), dtype=np.float32)
    for core in range(NCORES):
        b = core // (NCORES // B)
        y[b] += res.results[core]["y"]
        y[b, (NT - 1) * 512:] += res.results[core]["y3a"]
    y += np.asarray(b_out, dtype=np.float32)[None, None, :]
    return y
